# revision 1
# baseline (speedup 1.0000x reference)
"""Distributed Trainium2 Bass kernel for the AGCN (gnn_message_passing) problem.

Strategy (8 NeuronCores, SPMD):
  - Nodes partitioned by graph id: core c owns graphs [8c, 8c+8) and their
    nodes (batch is sorted, so per-core nodes are contiguous in the input).
    Within a core, nodes are reordered to balance per-128-node-block
    in-degree (evens out scatter-matmul tile counts).
  - Edges (incl. appended self-loops carrying the self_norm term) are owned
    by the dst node's core, grouped by dst block, padded to T tiles of 128.
  - Per layer: each core computes hW = feat @ W for its own nodes (PE),
    AllGathers the bf16 [R,1024] shard into a shared [8R,1024] DRAM table,
    then per dst block: dma_gather of the src rows (edge-major [128e, 1024])
    and a TensorE scatter-matmul  psum += S_tile^T @ msg  where S_tile is a
    host-built [128 edges, 128 nodes] one-hot*enorm matrix. Bias is added
    via a K=1 outer-product matmul. Epilogue: relu (+ residual) on DVE/ACT.
  - Pooling: per-graph max via a local gather into fixed 384-row graph
    slots, PE transposes to feature-major, DVE max-reduce.
  - Readout MLP runs per-core on its own 8 graphs; output rows are the
    core's graphs, host concatenates.
All matmul operands bf16 (fp32 PSUM accumulation); rel err ~1e-4.
"""
import os
import sys

for _p in ("/opt/trn_rl_repo",):
    if os.path.isdir(_p) and _p not in sys.path:
        sys.path.insert(0, _p)

import numpy as np
import ml_dtypes

from concourse import bass, bacc, tile
import concourse.mybir as mybir
from concourse.bass_utils import run_bass_kernel_spmd

BF = ml_dtypes.bfloat16
NCORES = 8
D = 512
DE = 1280
NEMB = 21


# ---------------------------------------------------------------------------
# host-side preprocessing
# ---------------------------------------------------------------------------

def _wrap_idx(a):
    """[n] int16 -> [128, n//16] wrapped (idx i at partition i%16, col i//16),
    replicated across the 8 Q7 core groups."""
    a = np.ascontiguousarray(np.asarray(a, np.int16)).reshape(-1, 16).T
    return np.ascontiguousarray(np.tile(a, (8, 1)))


def _prep(inputs):
    native_x = np.asarray(inputs["native_x"], np.int64)
    x = np.asarray(inputs["x"], np.float32)
    edge_index = np.asarray(inputs["edge_index"], np.int64)
    batch = np.asarray(inputs["batch"], np.int64)
    N = native_x.shape[0]
    G = 64
    gpc = G // NCORES

    src, dst = edge_index[0], edge_index[1]
    deg = np.bincount(dst, minlength=N).astype(np.float32) + 1.0
    dis = deg ** -0.5
    enorm = (dis[src] * dis[dst]).astype(np.float32)
    self_norm = (dis * dis).astype(np.float32)

    core_of_node = batch // gpc
    counts = np.bincount(core_of_node, minlength=NCORES)
    R = int(np.ceil(counts.max() / 128) * 128)
    B = R // 128

    gsizes = np.bincount(batch, minlength=G)
    assert gsizes.min() > 0, "empty graph unsupported"
    SLOT = int(np.ceil(gsizes.max() / 128) * 128)
    SB = gpc * SLOT // 128  # slot blocks per core

    indeg = np.bincount(dst, minlength=N) + 1

    # within-core node ordering: greedy in-degree balancing across B blocks
    loc = np.full(N, -1, np.int64)
    core_nodes = []
    for c in range(NCORES):
        nodes = np.nonzero(core_of_node == c)[0]
        core_nodes.append(nodes)
        order = nodes[np.argsort(-indeg[nodes], kind="stable")]
        load = np.zeros(B, np.int64)
        nfill = np.zeros(B, np.int64)
        for n in order:
            open_b = np.nonzero(nfill < 128)[0]
            bsel = open_b[np.argmin(load[open_b])]
            loc[n] = bsel * 128 + nfill[bsel]
            nfill[bsel] += 1
            load[bsel] += indeg[n]
    pg = core_of_node * R + loc
    assert NCORES * R < 32768

    # edges incl self loops, grouped by (dst core, dst block)
    all_src = np.concatenate([src, np.arange(N)])
    all_dst = np.concatenate([dst, np.arange(N)])
    all_w = np.concatenate([enorm, self_norm]).astype(np.float32)
    e_core = core_of_node[all_dst]
    e_block = loc[all_dst] // 128

    key = e_core * B + e_block
    sort = np.argsort(key, kind="stable")
    skey = key[sort]
    starts = np.searchsorted(skey, np.arange(NCORES * B))
    ends = np.searchsorted(skey, np.arange(NCORES * B) + 1)
    T = int(np.ceil((ends - starts).max() / 128))

    cores = []
    for c in range(NCORES):
        gidx = np.zeros((B, T * 128), np.int16)
        S = np.zeros((B, T * 128, 128), BF)
        for b in range(B):
            sl = sort[starts[c * B + b]:ends[c * B + b]]
            k = len(sl)
            if k:
                gidx[b, :k] = pg[all_src[sl]].astype(np.int16)
                S[b, np.arange(k), loc[all_dst[sl]] % 128] = all_w[sl].astype(BF)
        nodes = core_nodes[c]
        xT = np.zeros((DE, R), np.float32)
        xT[:, loc[nodes]] = x[nodes].T
        oh = np.zeros((NEMB, R), BF)
        oh[native_x[nodes], loc[nodes]] = 1.0
        sidx = np.zeros(gpc * SLOT, np.int64)
        for j in range(gpc):
            g = c * gpc + j
            gn = nodes[batch[nodes] == g]
            sidx[j * SLOT:(j + 1) * SLOT] = loc[gn[0]]
            sidx[j * SLOT:j * SLOT + len(gn)] = loc[gn]

        # device layouts
        st_dev = np.ascontiguousarray(
            S.reshape(B, T, 128, 128).transpose(2, 0, 1, 3).reshape(128, B * T * 128))
        gidx_dev = np.concatenate([_wrap_idx(gidx[b]) for b in range(B)], axis=1)
        sidx_dev = _wrap_idx(sidx)
        xT_dev = np.ascontiguousarray(
            xT.reshape(10, 128, R).transpose(1, 0, 2).reshape(128, 10 * R)).astype(BF)
        cores.append(dict(stiles=st_dev, gidx=gidx_dev, sidx=sidx_dev,
                          xT=xT_dev, oh=np.ascontiguousarray(oh)))
    return dict(cores=cores, R=R, B=B, T=T, SLOT=SLOT, SB=SB, gpc=gpc)


def _params(inputs, dims):
    """Parameter tensors (identical on every core)."""
    emb = np.asarray(inputs["embed_table"], np.float32)
    aaw = np.asarray(inputs["proj_aa_w"], np.float32)
    R = dims["R"]
    p = {}
    p["aa_tab"] = np.ascontiguousarray((emb @ aaw).astype(BF))            # [21, 512]
    wesm = np.asarray(inputs["proj_esm_w"], np.float32).astype(BF)        # [1280,512]
    p["wesm"] = np.ascontiguousarray(
        wesm.reshape(10, 128, D).transpose(1, 0, 2).reshape(128, 10 * D))
    gw = np.asarray(inputs["gcn_w"], np.float32).astype(BF)               # [3,512,512]
    p["gcnw"] = np.ascontiguousarray(
        gw.reshape(3, 4, 128, D).transpose(2, 0, 1, 3).reshape(128, 12 * D))
    p["b_esm"] = np.asarray(inputs["proj_esm_b"], np.float32).astype(BF).reshape(1, D)
    p["b_aa"] = np.asarray(inputs["proj_aa_b"], np.float32).astype(BF).reshape(1, D)
    gb = np.asarray(inputs["gcn_b"], np.float32).astype(BF)
    p["gcnb"] = np.ascontiguousarray(np.tile(gb, (1, 2)).reshape(1, -1)) # [1, 3*1024]
    r1 = np.asarray(inputs["ro1_w"], np.float32).astype(BF)               # [512,1024]
    p["ro1w"] = np.ascontiguousarray(
        r1.reshape(4, 128, 1024).transpose(1, 0, 2).reshape(128, 4 * 1024))
    p["ro1b"] = np.asarray(inputs["ro1_b"], np.float32).astype(BF).reshape(1, 1024)
    r2 = np.asarray(inputs["ro2_w"], np.float32).astype(BF)               # [1024,500]
    p["ro2w"] = np.ascontiguousarray(
        r2.reshape(8, 128, 500).transpose(1, 0, 2).reshape(128, 8 * 500))
    p["ro2b"] = np.asarray(inputs["ro2_b"], np.float32).astype(BF).reshape(1, 500)
    w1 = np.asarray(inputs["weight1"], np.float32)
    p["w1a"] = np.full((128, 1), w1[0], np.float32)
    p["w1b"] = np.full((128, 1), w1[1], np.float32)
    p["ident"] = np.eye(128, dtype=BF)
    p["ident8"] = np.eye(8, dtype=BF)
    p["ones"] = np.ones((1, 128), BF)
    return p


# ---------------------------------------------------------------------------
# device kernel builder
# ---------------------------------------------------------------------------

def _build(dims):
    SKIP = set(os.environ.get("AGCN_SKIP", "").split(","))
    R, B, T, SLOT, SB, gpc = (dims["R"], dims["B"], dims["T"], dims["SLOT"],
                              dims["SB"], dims["gpc"])
    f32, bf16, i16 = mybir.dt.float32, mybir.dt.bfloat16, mybir.dt.int16
    RELU = mybir.ActivationFunctionType.Relu
    SIGM = mybir.ActivationFunctionType.Sigmoid

    nc = bacc.Bacc(None, target_bir_lowering=False, debug=False)
    lean = "leanio" in SKIP

    # I/O
    d_xT = lean or nc.declare_dram_parameter("xT", [128, 10 * R], bf16, isOutput=False)
    d_oh = lean or nc.declare_dram_parameter("oh", [NEMB, R], bf16, isOutput=False)
    d_st = nc.declare_dram_parameter("stiles", [128, B * T * 128], bf16, isOutput=False)
    d_gidx = nc.declare_dram_parameter("gidx", [128, B * T * 8], i16, isOutput=False)
    d_sidx = lean or nc.declare_dram_parameter("sidx", [128, gpc * SLOT // 16], i16, isOutput=False)
    d_aatab = lean or nc.declare_dram_parameter("aa_tab", [NEMB, D], bf16, isOutput=False)
    d_wesm = lean or nc.declare_dram_parameter("wesm", [128, 10 * D], bf16, isOutput=False)
    d_gcnw = lean or nc.declare_dram_parameter("gcnw", [128, 12 * D], bf16, isOutput=False)
    d_besm = lean or nc.declare_dram_parameter("b_esm", [1, D], bf16, isOutput=False)
    d_baa = lean or nc.declare_dram_parameter("b_aa", [1, D], bf16, isOutput=False)
    d_gcnb = lean or nc.declare_dram_parameter("gcnb", [1, 3 * 1024], bf16, isOutput=False)
    d_ro1w = lean or nc.declare_dram_parameter("ro1w", [128, 4 * 1024], bf16, isOutput=False)
    d_ro1b = lean or nc.declare_dram_parameter("ro1b", [1, 1024], bf16, isOutput=False)
    d_ro2w = lean or nc.declare_dram_parameter("ro2w", [128, 8 * 500], bf16, isOutput=False)
    d_ro2b = lean or nc.declare_dram_parameter("ro2b", [1, 500], bf16, isOutput=False)
    d_w1a = lean or nc.declare_dram_parameter("w1a", [128, 1], f32, isOutput=False)
    d_w1b = lean or nc.declare_dram_parameter("w1b", [128, 1], f32, isOutput=False)
    d_ident = lean or nc.declare_dram_parameter("ident", [128, 128], bf16, isOutput=False)
    d_ident8 = lean or nc.declare_dram_parameter("ident8", [8, 8], bf16, isOutput=False)
    d_ones = lean or nc.declare_dram_parameter("ones", [1, 128], bf16, isOutput=False)
    OUTP = 128 if "bigout" in SKIP else gpc
    d_out = nc.declare_dram_parameter("out", [OUTP, 500], f32, isOutput=True)

    # internal DRAM
    shard = nc.dram_tensor("hw_shard", [R, 1024], bf16)
    shared_tab = os.environ.get("AGCN_LOCAL_TABLE", "") == ""
    tables = [nc.dram_tensor(f"table{l}", [NCORES * R, 1024], bf16,
                             addr_space="Shared" if shared_tab else "Local")
              for l in range(3)]
    ffin = nc.dram_tensor("feat_final", [R, 1024], bf16)

    with tile.TileContext(nc) as tc:
        with (
            tc.tile_pool(name="persist", bufs=1) as pers,
            tc.tile_pool(name="feat", bufs=1) as featp,
        ):
            # persistent params in SBUF
            s_st = pers.tile([128, B * T * 128], bf16)
            nc.sync.dma_start(s_st[:], d_st.ap())
            s_gidx = pers.tile([128, B * T * 8], i16)
            nc.sync.dma_start(s_gidx[:], d_gidx.ap())
            s_gcnw = pers.tile([128, 12, D], bf16)
            if "nopers" not in SKIP:
                nc.sync.dma_start(s_gcnw[:], d_gcnw.ap().rearrange("p (a d) -> p a d", d=D))
            s_gcnb = pers.tile([1, 3, 1024], bf16)
            if "nopers" not in SKIP:
                nc.sync.dma_start(s_gcnb[:], d_gcnb.ap().rearrange("p (a d) -> p a d", d=1024))
            s_ident = pers.tile([128, 128], bf16)
            if "nopers" not in SKIP:
                nc.sync.dma_start(s_ident[:], d_ident.ap())
            s_ones = pers.tile([1, 128], bf16)
            if "nopers" not in SKIP:
                nc.sync.dma_start(s_ones[:], d_ones.ap())

            featA = featp.tile([128, B, D], bf16, tag="featA")
            featB = featp.tile([128, B, D], bf16, tag="featB")

            # ---------------- input projection ----------------
            with (
                tc.tile_pool(name="xin", bufs=3) as xin,
                tc.tile_pool(name="prm1", bufs=1) as prm1,
                tc.tile_pool(name="ppsum", bufs=2, space="PSUM") as ppsum,
                tc.tile_pool(name="ppsum2", bufs=2, space="PSUM") as ppsum2,
            ):
                if "noprm" in SKIP:
                    s_wesm = s_oh = s_aatab = s_besm = s_baa = None
                s_wesm = prm1.tile([128, 10, D], bf16)
                if "noprm" not in SKIP:
                    nc.sync.dma_start(s_wesm[:], d_wesm.ap().rearrange("p (a d) -> p a d", d=D))
                s_oh = prm1.tile([NEMB, R], bf16)
                if "noprm" not in SKIP:
                    nc.sync.dma_start(s_oh[:], d_oh.ap())
                s_aatab = prm1.tile([NEMB, D], bf16)
                if "noprm" not in SKIP:
                    nc.sync.dma_start(s_aatab[:], d_aatab.ap())
                s_besm = prm1.tile([1, D], bf16)
                if "noprm" not in SKIP:
                    nc.sync.dma_start(s_besm[:], d_besm.ap())
                s_baa = prm1.tile([1, D], bf16)
                if "noprm" not in SKIP:
                    nc.sync.dma_start(s_baa[:], d_baa.ap())

                xT3 = None if lean else d_xT.ap().rearrange("p (a r) -> p a r", r=R)
                if "inproj" in SKIP and "nofeat" not in SKIP:
                    nc.vector.memset(featA[:], 0.125)
                    nc.vector.memset(featB[:], 0.125)
                for b in (range(B) if "inproj" not in SKIP else range(0)):
                    xt = xin.tile([128, 10, 128], bf16, tag="xt")
                    nc.sync.dma_start(xt[:], xT3[:, :, b * 128:(b + 1) * 128])
                    ps1 = ppsum.tile([128, D], f32, tag="ps1")
                    for k in range(10):
                        nc.tensor.matmul(ps1[:], xt[:, k, :], s_wesm[:, k, :],
                                         start=(k == 0), stop=False)
                    nc.tensor.matmul(ps1[:], s_ones[:], s_besm[:],
                                     start=False, stop=True)
                    ps2 = ppsum2.tile([128, D], f32, tag="ps2")
                    nc.tensor.matmul(ps2[:], s_oh[:, b * 128:(b + 1) * 128],
                                     s_aatab[:], start=True, stop=False)
                    nc.tensor.matmul(ps2[:], s_ones[:], s_baa[:],
                                     start=False, stop=True)
                    lin = xin.tile([128, D], f32, tag="lin")
                    nc.vector.tensor_copy(lin[:], ps1[:])
                    nc.scalar.activation(featB[:, b, :], ps1[:], RELU)
                    nc.vector.tensor_add(ps2[:], ps2[:], lin[:])
                    nc.scalar.activation(featA[:, b, :], ps2[:], RELU)

            # ---------------- GCN layers ----------------
            st3 = s_st[:].rearrange("p (bt n) -> p bt n", n=128)
            for l in range(3):
                with (
                    tc.tile_pool(name=f"ft{l}", bufs=3) as ftp,
                    tc.tile_pool(name=f"hw{l}", bufs=3) as hwp,
                    tc.tile_pool(name=f"gb{l}", bufs=3) as gbp,
                    tc.tile_pool(name=f"ep{l}", bufs=3) as epp,
                    tc.tile_pool(name=f"pst{l}", bufs=2, space="PSUM") as pst,
                    tc.tile_pool(name=f"psh{l}", bufs=2, space="PSUM") as psh,
                    tc.tile_pool(name=f"psa{l}", bufs=2, space="PSUM") as psa,
                ):
                    # hW shard: feat @ W_l, written to DRAM + AllGather
                    for b in range(B):
                        for s, feat in ((0, featA), (1, featB)):
                            if "hw" in SKIP:
                                if "noshard" not in SKIP:
                                    hw = hwp.tile([128, D], bf16, tag="hw")
                                    nc.vector.memset(hw[:], 0.5)
                                    nc.sync.dma_start(
                                        shard.ap()[b * 128:(b + 1) * 128,
                                                   s * D:(s + 1) * D], hw[:])
                                continue
                            ftps = pst.tile([128, D], f32, tag="ftps")
                            for k in range(4):
                                nc.tensor.matmul(
                                    ftps[:, k * 128:(k + 1) * 128],
                                    feat[:, b, k * 128:(k + 1) * 128],
                                    s_ident[:], start=True, stop=True)
                            ft = ftp.tile([128, D], bf16, tag="ft")
                            nc.vector.tensor_copy(ft[:], ftps[:])
                            hwps = psh.tile([128, D], f32, tag="hwps")
                            for k in range(4):
                                nc.tensor.matmul(hwps[:], ft[:, k * 128:(k + 1) * 128],
                                                 s_gcnw[:, l * 4 + k, :],
                                                 start=(k == 0), stop=(k == 3))
                            hw = hwp.tile([128, D], bf16, tag="hw")
                            nc.vector.tensor_copy(hw[:], hwps[:])
                            nc.sync.dma_start(
                                shard.ap()[b * 128:(b + 1) * 128,
                                           s * D:(s + 1) * D], hw[:])
                    if "cc" not in SKIP:
                        nc.gpsimd.collective_compute(
                            "AllGather", mybir.AluOpType.bypass,
                            replica_groups=[list(range(NCORES))],
                            ins=[shard.ap().opt()],
                            outs=[tables[l].ap().opt()],
                        )
                    # gather + scatter-matmul + epilogue per dst block
                    for b in range(B):
                        gbuf = gbp.tile([128, T, 1024], bf16, tag="gbuf")
                        if "gather" in SKIP:
                            nc.vector.memset(gbuf[:], 0.25)
                        else:
                            # dma_gather calls with num_idxs > 1024 wedge the
                            # device -- split into <=8-tile (1024-idx) calls.
                            for t0 in range(0, T, 8):
                                nt = min(8, T - t0)
                                nc.gpsimd.dma_gather(
                                    gbuf[:, t0:t0 + nt, :], tables[l].ap(),
                                    s_gidx[:, (b * T + t0) * 8:(b * T + t0 + nt) * 8],
                                    nt * 128, nt * 128, 1024)
                        aps = psa.tile([128, 1024], f32, tag="aps")
                        TL = list(range(T) if "scatter" not in SKIP else range(1))
                        if "nomm" in SKIP:
                            TL = []
                        skip_bias = "bias" in SKIP
                        for t in TL:
                            for h in range(2):
                                nc.tensor.matmul(
                                    aps[:, h * D:(h + 1) * D],
                                    st3[:, b * T + t, :],
                                    gbuf[:, t, h * D:(h + 1) * D],
                                    start=(t == 0),
                                    stop=(skip_bias and t == TL[-1]))
                        if not skip_bias:
                            for h in range(2):
                                nc.tensor.matmul(aps[:, h * D:(h + 1) * D],
                                                 s_ones[:],
                                                 s_gcnb[:, l, h * D:(h + 1) * D],
                                                 start=False, stop=True)
                        if "noepi2" in SKIP:
                            pass
                        elif "epi" in SKIP:
                            nc.vector.memset(featA[:, b, :], 0.25)
                            nc.vector.memset(featB[:, b, :], 0.25)
                        elif l == 0:
                            nc.scalar.activation(featA[:, b, :], aps[:, :D], RELU)
                            nc.scalar.activation(featB[:, b, :], aps[:, D:], RELU)
                        else:
                            ra = epp.tile([128, D], bf16, tag="ra")
                            rb = epp.tile([128, D], bf16, tag="rb")
                            nc.scalar.activation(ra[:], aps[:, :D], RELU)
                            nc.scalar.activation(rb[:], aps[:, D:], RELU)
                            nc.vector.tensor_add(featA[:, b, :], featA[:, b, :], ra[:])
                            nc.vector.tensor_add(featB[:, b, :], featB[:, b, :], rb[:])

            # ---------------- pooling + readout ----------------
            if "pool" in SKIP:
                with tc.tile_pool(name="yp", bufs=1) as yp:
                    y = yp.tile([OUTP, 500], f32)
                    nc.vector.memset(y[:], 0.5)
                    nc.sync.dma_start(d_out.ap(), y[:])
            if "pool" not in SKIP:
             with (
                tc.tile_pool(name="pool", bufs=1) as poolp,
                tc.tile_pool(name="slot", bufs=3) as slotp,
                tc.tile_pool(name="prm2", bufs=1) as prm2,
                tc.tile_pool(name="pps", bufs=2, space="PSUM") as pps,
                tc.tile_pool(name="rps", bufs=1, space="PSUM") as rps,
             ):
                for b in range(B):
                    nc.sync.dma_start(ffin.ap()[b * 128:(b + 1) * 128, :D],
                                      featA[:, b, :])
                    nc.sync.dma_start(ffin.ap()[b * 128:(b + 1) * 128, D:],
                                      featB[:, b, :])
                s_sidx = prm2.tile([128, gpc * SLOT // 16], i16)
                nc.sync.dma_start(s_sidx[:], d_sidx.ap())
                poolT = poolp.tile([128, SB, 8, 128], bf16)
                slot3 = gpc * SLOT // 128
                assert slot3 == SB
                for sb0 in range(0, SB, 3):
                    nsb = min(3, SB - sb0)
                    sbuf = slotp.tile([128, 3, 1024], bf16, tag="sbuf")
                    nc.gpsimd.dma_gather(
                        sbuf[:, :nsb, :], ffin.ap(),
                        s_sidx[:, sb0 * 8:(sb0 + nsb) * 8],
                        nsb * 128, nsb * 128, 1024)
                    for j in range(nsb):
                        for kh in range(2):
                            tps = pps.tile([128, D], f32, tag="tps")
                            for k in range(4):
                                nc.tensor.matmul(
                                    tps[:, k * 128:(k + 1) * 128],
                                    sbuf[:, j, (kh * 4 + k) * 128:(kh * 4 + k + 1) * 128],
                                    s_ident[:], start=True, stop=True)
                            nc.vector.tensor_copy(
                                poolT[:, sb0 + j, kh * 4:(kh + 1) * 4, :], tps[:])
                # per-graph max over slots
                spg = SLOT // 128
                pooled = prm2.tile([128, 8, gpc], f32)
                for g in range(gpc):
                    for kb in range(8):
                        nc.vector.tensor_reduce(
                            pooled[:, kb, g:g + 1],
                            poolT[:, g * spg:(g + 1) * spg, kb, :],
                            axis=mybir.AxisListType.XY,
                            op=mybir.AluOpType.max)
                s_w1a = prm2.tile([128, 1], f32)
                nc.sync.dma_start(s_w1a[:], d_w1a.ap())
                s_w1b = prm2.tile([128, 1], f32)
                nc.sync.dma_start(s_w1b[:], d_w1b.ap())
                gT = prm2.tile([128, 4, gpc], bf16)
                gtmp = prm2.tile([128, 4, gpc], f32)
                gtmp2 = prm2.tile([128, 4, gpc], f32)
                nc.vector.tensor_scalar(gtmp[:], pooled[:, 0:4, :], s_w1a[:], None,
                                        mybir.AluOpType.mult)
                nc.vector.tensor_scalar(gtmp2[:], pooled[:, 4:8, :], s_w1b[:], None,
                                        mybir.AluOpType.mult)
                nc.vector.tensor_add(gT[:], gtmp[:], gtmp2[:])

                s_ro1w = prm2.tile([128, 4, 1024], bf16)
                nc.sync.dma_start(s_ro1w[:], d_ro1w.ap().rearrange("p (a d) -> p a d", d=1024))
                s_ro1b = prm2.tile([1, 1024], bf16)
                nc.sync.dma_start(s_ro1b[:], d_ro1b.ap())
                s_ro2w = prm2.tile([128, 8, 500], bf16)
                nc.sync.dma_start(s_ro2w[:], d_ro2w.ap().rearrange("p (a d) -> p a d", d=500))
                s_ro2b = prm2.tile([1, 500], bf16)
                nc.sync.dma_start(s_ro2b[:], d_ro2b.ap())
                s_id8 = prm2.tile([8, 8], bf16)
                nc.sync.dma_start(s_id8[:], d_ident8.ap())

                r1ps = rps.tile([gpc, 1024], f32)
                for h in range(2):
                    for k in range(4):
                        nc.tensor.matmul(r1ps[:, h * D:(h + 1) * D],
                                         gT[:, k, :],
                                         s_ro1w[:, k, h * D:(h + 1) * D],
                                         start=(k == 0), stop=False)
                    nc.tensor.matmul(r1ps[:, h * D:(h + 1) * D],
                                     s_ones[:, :gpc],
                                     s_ro1b[:, h * D:(h + 1) * D],
                                     start=False, stop=True)
                h1 = prm2.tile([gpc, 1024], bf16)
                nc.scalar.activation(h1[:], r1ps[:], RELU)
                h1T = prm2.tile([128, 8, gpc], bf16)
                for k in range(8):
                    tps = pps.tile([128, gpc], f32, tag="tps8")
                    nc.tensor.matmul(tps[:], h1[:, k * 128:(k + 1) * 128],
                                     s_id8[:], start=True, stop=True)
                    nc.vector.tensor_copy(h1T[:, k, :], tps[:])
                yps = rps.tile([gpc, 500], f32)
                for k in range(8):
                    nc.tensor.matmul(yps[:], h1T[:, k, :], s_ro2w[:, k, :],
                                     start=(k == 0), stop=False)
                nc.tensor.matmul(yps[:], s_ones[:, :gpc], s_ro2b[:],
                                 start=False, stop=True)
                y = prm2.tile([gpc, 500], f32)
                nc.scalar.activation(y[:], yps[:], SIGM)
                nc.sync.dma_start(d_out.ap(), y[:])

    nc.compile()
    return nc


# ---------------------------------------------------------------------------
# entry point
# ---------------------------------------------------------------------------

_CACHE = {}


def kernel(**inputs):
    pp = _prep(inputs)
    dims = {k: pp[k] for k in ("R", "B", "T", "SLOT", "SB", "gpc")}
    key = tuple(sorted(dims.items()))
    if key not in _CACHE:
        _CACHE[key] = _build(dims)
    nc = _CACHE[key]
    par = _params(inputs, dims)
    in_maps = []
    for c in range(NCORES):
        m = dict(par)
        m.update(pp["cores"][c])
        in_maps.append(m)
    res = run_bass_kernel_spmd(nc, in_maps, core_ids=list(range(NCORES)))
    out = np.concatenate([res.results[c]["out"] for c in range(NCORES)], 0)
    return out.astype(np.float32)

